# revision 7
# baseline (speedup 1.0000x reference)
"""DLRM dot-interaction kernel for Trainium2 (8 NeuronCores, batch-sharded).

Per sample b: T = concat(dense[b], embs[b]) -> [27, 128]; Z = T @ T^T;
output = strict upper triangle of Z -> [351] fp32.

Per-core plan (2048 samples, 16 blocks of 128):
  - SWDGE cast-DMA loads 4 blocks at a time as [128 b, (f,d)] fp16.
  - PE transposes each [128 b, 128 d] feature slab into PSUM; DVE/ACT copy
    into f-major Tt [128 d, f*128+b] fp16 (contiguous copies).
  - Per-sample fp16 matmul: lhsT = [128 d, 32 f] (27 + 5 zero pads), rhs =
    [128 d, 27 f]; out -> PSUM zp [32*(g) + m, q*32 + n] fp32 (col-group
    tiling, sample s = q*4 + g).  Transposes of block b+1 are interleaved
    with the gram matmuls of block b at ~1us grain so the PE HAM clock
    stays at 8/8 (gram bursts give busy credit through transpose phases).
  - One DVE StreamTranspose per block swaps m<->q inside each 32-partition
    quadrant (g stays put): PSUM [(g,m), (n,q)-view] -> SBUF Zb
    [(g,q), m*32+n] fp32.  This replaces the baseline's DRAM scratch
    bounce + 55k-descriptor gather entirely.
  - Triu pack: 26 contiguous-run DVE/ACT copies per 4-block group into
    Pk [(g,q), t*351], then one HWDGE DMA with 1404B runs writes
    out[b, :] (partition (g,q) -> row q*4+g).
"""

import numpy as np

B, NUM_EMBS, D = 16384, 26, 128
N_CORES = 8
BC = B // N_CORES  # 2048 samples per core
BLK = 128          # samples per block
NF = NUM_EMBS + 1  # 27 features
FP = 32            # feature pitch (27 + 5 pad)
NPAIR = NF * (NF - 1) // 2  # 351
PACK = 4           # blocks per pack/output group

_CACHE = {}


def build(bc=BC):
    import concourse.bacc as bacc
    import concourse.mybir as mybir
    from concourse.tile import TileContext
    from concourse.masks import make_identity

    fp16 = mybir.dt.float16
    fp32 = mybir.dt.float32

    nc = bacc.Bacc("TRN2", target_bir_lowering=False, debug=False)
    dense_t = nc.dram_tensor("dense", (bc, D), fp32, kind="ExternalInput")
    embs_t = nc.dram_tensor("embs", (bc, NUM_EMBS, D), fp32, kind="ExternalInput")
    out_t = nc.dram_tensor("out", (bc, NPAIR), fp32, kind="ExternalOutput")

    nblk = bc // BLK
    assert nblk % PACK == 0

    # Input load plan: small groups first (fast pipeline start), 4-block
    # groups at steady state (fewer SWDGE generations).
    groups = []
    b = 0
    head = [1, 1, 2]
    while b < nblk:
        sz = min(head.pop(0) if head else 4, nblk - b)
        groups.append((b, sz))
        b += sz
    g_of = {}
    for gs, sz in groups:
        for i in range(sz):
            g_of[gs + i] = (gs, sz)

    with TileContext(nc) as tc:
        with (
            tc.tile_pool(name="consts", bufs=1) as consts,
            tc.tile_pool(name="xin", bufs=2) as xpool,
            tc.tile_pool(name="tt", bufs=3) as ttpool,
            tc.tile_pool(name="zb", bufs=2) as zbpool,
            tc.tile_pool(name="pk", bufs=2) as pkpool,
            tc.tile_pool(name="tp", bufs=2, space="PSUM") as tppool,
            tc.tile_pool(name="zp", bufs=2, space="PSUM") as zppool,
        ):
            ident = consts.tile([128, 128], fp16)
            make_identity(nc, ident)

            dview = dense_t.ap()  # [bc, 128]
            eview = embs_t.ap().rearrange("b f d -> b (f d)")  # [bc, 3328]
            oview = out_t.ap()  # [bc, 351]

            X = None
            state = {}  # per-block: (Tt tile,), (zp tile,)
            zb_t = None

            def emit_load(blk):
                nonlocal X
                gs, gsz = g_of[blk]
                if blk != gs:
                    return
                X = xpool.tile([BLK, gsz * NF * D], fp16, tag="X")
                dsrc = dview[gs * BLK : (gs + gsz) * BLK].rearrange(
                    "(t b) d -> b t d", t=gsz
                )  # [128, gsz, 128]
                xd = X.rearrange("b (t c) -> b t c", t=gsz)
                nc.gpsimd.dma_start(out=xd[:, :, 0:D], in_=dsrc)
                esrc = eview[gs * BLK : (gs + gsz) * BLK].rearrange(
                    "(t b) c -> b t c", t=gsz
                )  # [128, gsz, 3328]
                nc.gpsimd.dma_start(out=xd[:, :, D:], in_=esrc)

            def emit_transpose_chunk(blk, ci, Tt):
                """Chunk ci of the b->d transposes for block blk.

                Expressed as regular matmuls (out = X_f.T @ I) rather than
                transpose-mode: numerically identical, but regular MMs feed
                the PE HAM activity monitor (transpose-mode does not), so
                the array stays at the full 2.4 GHz clock, and the 128-col
                fp16 contiguous weight load is FWL-eligible."""
                gs, gsz = g_of[blk]
                xoff = (blk - gs) * NF * D
                c0 = ci * 8
                cf = min(8, NF - c0)
                tp = tppool.tile([128, 8 * BLK], fp32, tag="tp")
                for j in range(cf):
                    f = c0 + j
                    nc.tensor.matmul(
                        tp[:, j * BLK : (j + 1) * BLK],
                        X[:, xoff + f * D : xoff + (f + 1) * D],
                        ident,
                        start=True,
                        stop=True,
                    )
                dst = Tt[:, c0 * BLK : (c0 + cf) * BLK]
                src = tp[:, : cf * BLK]
                # fp32 -> fp16 cast in the copy; split DVE/ACT evenly
                if ci % 2 == 0:
                    nc.vector.tensor_copy(out=dst, in_=src)
                else:
                    nc.scalar.copy(dst, src)

            def emit_gram_chunk(blk, ci, Tt, zp):
                """q-groups [8*ci, 8*ci+8) of the per-sample gram matmuls."""
                Ttr = Tt.rearrange("d (f b) -> d b f", b=BLK)
                for q in range(8 * ci, 8 * ci + 8):
                    for g in range(4):
                        s = q * 4 + g
                        nc.tensor.matmul(
                            zp[32 * g : 32 * g + NF, q * FP : q * FP + NF],
                            Ttr[:, s, :],       # [128 d, 27 f] weights
                            Ttr[:, s, :],       # [128 d, 27 f] moving
                            start=True,
                            stop=True,
                            tile_position=(0, 32 * g),
                        )

            def emit_ztranspose(blk, zp):
                """PSUM [(g,m), (q,n)] -> Zb[(g,q), t, m*32+n] via 32x32
                DVE block transposes (m<->q swap within each quadrant)."""
                t = blk % PACK
                inv = zp.rearrange("p (q n) -> p n q", n=FP)[:, 0:NF, :]
                outv = zb_t.rearrange("p (t m n) -> p t n m", t=PACK, n=FP)[
                    :, t, 0:NF, :
                ]
                nc.vector.transpose(out=outv, in_=inv)

            def emit_pack_out(qtr):
                zbp = zb_t.rearrange("p (t m n) -> p t m n", t=PACK, n=FP)
                Pk = pkpool.tile([128, PACK * NPAIR], fp32, tag="Pk")
                pkp = Pk.rearrange("p (t c) -> p t c", t=PACK)
                off = 0
                for m in range(NF - 1):
                    ln = NF - 1 - m
                    src = zbp[:, :, m, m + 1 : NF]  # [128, t, ln]
                    dst = pkp[:, :, off : off + ln]
                    if m % 2 == 0:
                        nc.vector.tensor_copy(out=dst, in_=src)
                    else:
                        nc.scalar.copy(dst, src)
                    off += ln
                b0 = qtr * PACK * BLK
                ovq = oview[b0 : b0 + PACK * BLK].rearrange(
                    "(t q g) c -> g q t c", t=PACK, g=4
                )  # [4, 32, t, 351]
                pk4 = pkp.rearrange("(g q) t c -> g q t c", g=4)
                for g in range(4):
                    eng = nc.sync if g % 2 == 0 else nc.scalar
                    eng.dma_start(out=ovq[g], in_=pk4[g])

            # Software pipeline: transposes of block b+1 interleave with the
            # gram matmuls of block b (chunk granularity ~1us on PE).
            for blk in range(nblk + 1):
                Tt = None
                if blk < nblk:
                    emit_load(blk)
                    Tt = ttpool.tile([128, NF * D], fp16, tag="Tt")
                if blk >= 1:
                    pb = blk - 1
                    zp = zppool.tile([128, FP * FP], fp32, tag="zp")
                    state[pb] = (state[pb][0], zp)
                    if pb % PACK == 0:
                        zb_t = zbpool.tile([128, PACK * FP * FP], fp32, tag="Zb")
                for ci in range(4):
                    if blk < nblk:
                        emit_transpose_chunk(blk, ci, Tt)
                    if blk >= 1:
                        pb = blk - 1
                        emit_gram_chunk(pb, ci, state[pb][0], state[pb][1])
                if blk < nblk:
                    state[blk] = (Tt, None)
                if blk >= 1:
                    pb = blk - 1
                    emit_ztranspose(pb, state[pb][1])
                    if pb % PACK == PACK - 1:
                        emit_pack_out(pb // PACK)
                    del state[pb]

    nc.compile()
    return nc


def _get(bc=BC):
    if bc not in _CACHE:
        _CACHE[bc] = build(bc)
    return _CACHE[bc]


def kernel(dense: np.ndarray, embs: np.ndarray) -> np.ndarray:
    from concourse import bass_utils

    dense = np.ascontiguousarray(np.asarray(dense, dtype=np.float32))
    embs = np.ascontiguousarray(np.asarray(embs, dtype=np.float32))
    assert dense.shape == (B, D) and embs.shape == (B, NUM_EMBS, D)

    nc = _get()
    dsh = dense.reshape(N_CORES, BC, D)
    esh = embs.reshape(N_CORES, BC, NUM_EMBS, D)
    in_maps = [{"dense": dsh[i], "embs": esh[i]} for i in range(N_CORES)]
    res = bass_utils.run_bass_kernel_spmd(nc, in_maps, core_ids=list(range(N_CORES)))
    return np.concatenate([r["out"] for r in res.results], axis=0)


# revision 9
# speedup vs baseline: 1.2513x; 1.2513x over previous
"""DLRM dot-interaction kernel for Trainium2 (8 NeuronCores, batch-sharded).

Per sample b: T = concat(dense[b], embs[b]) -> [27, 128]; Z = T @ T^T;
output = strict upper triangle of Z -> [351] fp32.

Per-core plan (2048 samples, 16 blocks of 128):
  - SWDGE cast-DMA loads 4 blocks at a time as [128 b, (f,d)] fp16.
  - Big-phase structure in groups of 8 blocks: first a transpose phase
    (216 PE transpose-mode matmuls [128 b, 128 d] -> PSUM fp16, DVE/ACT
    copy into f-major Tt [128 d, f*128+b] fp16), then one dense gram
    mega-burst (1024 back-to-back per-sample matmuls, ~26us of
    uninterrupted PE work) so the PE HAM activity monitor reaches and
    holds the full 2.4 GHz clock; short interleaved bursts never warm up.
  - Per-sample gram: lhsT = rhs = [128 d, 27 f] strided slice of Tt;
    out -> PSUM zp [32*g + m, q*32 + n] fp32 (col-group tiling,
    sample s = q*4 + g; 4 col groups run concurrently).
  - One DVE StreamTranspose per block swaps m<->q inside each 32-partition
    quadrant (g stays put): PSUM [(g,m), (n,q)-view] -> SBUF Zb
    [(g,q), m*32+n] fp32.  This replaces a DRAM scratch bounce + gather
    (55k small DMA descriptors) with 16 DVE instructions.
  - Triu pack: 26 contiguous-run DVE/ACT copies per 4-block group into
    Pk [(g,q), t*351], then HWDGE DMAs with 1404B runs write out[b, :]
    (partition (g,q) -> row q*4+g).
"""

import numpy as np

B, NUM_EMBS, D = 16384, 26, 128
N_CORES = 8
BC = B // N_CORES  # 2048 samples per core
BLK = 128          # samples per block
NF = NUM_EMBS + 1  # 27 features
FP = 32            # feature pitch in the Z PSUM tile
NPAIR = NF * (NF - 1) // 2  # 351
PACK = 4           # blocks per pack/output group
PHASE = 8          # blocks per transpose/gram phase pair

_CACHE = {}


def build(bc=BC):
    import concourse.bacc as bacc
    import concourse.mybir as mybir
    from concourse.tile import TileContext
    from concourse.masks import make_identity

    fp16 = mybir.dt.float16
    fp32 = mybir.dt.float32

    nc = bacc.Bacc("TRN2", target_bir_lowering=False, debug=False)
    dense_t = nc.dram_tensor("dense", (bc, D), fp32, kind="ExternalInput")
    embs_t = nc.dram_tensor("embs", (bc, NUM_EMBS, D), fp32, kind="ExternalInput")
    out_t = nc.dram_tensor("out", (bc, NPAIR), fp32, kind="ExternalOutput")

    nblk = bc // BLK
    assert nblk % PHASE == 0 and PHASE % PACK == 0

    # Input load groups of 4 blocks (one small head group for fast start).
    groups = []
    b = 0
    head = [1, 3]
    while b < nblk:
        sz = min(head.pop(0) if head else 4, nblk - b)
        groups.append((b, sz))
        b += sz
    g_of = {}
    for gs, sz in groups:
        for i in range(sz):
            g_of[gs + i] = (gs, sz)

    with TileContext(nc) as tc:
        with (
            tc.tile_pool(name="consts", bufs=1) as consts,
            tc.tile_pool(name="xin", bufs=3) as xpool,
            tc.tile_pool(name="tt", bufs=PHASE + 1) as ttpool,
            tc.tile_pool(name="zb", bufs=2) as zbpool,
            tc.tile_pool(name="pk", bufs=2) as pkpool,
            tc.tile_pool(name="tp", bufs=2, space="PSUM") as tppool,
            tc.tile_pool(name="zp", bufs=3, space="PSUM") as zppool,
        ):
            ident = consts.tile([128, 128], fp16)
            make_identity(nc, ident)

            dview = dense_t.ap()  # [bc, 128]
            eview = embs_t.ap().rearrange("b f d -> b (f d)")  # [bc, 3328]
            oview = out_t.ap()  # [bc, 351]

            X = None
            xmap = {}  # blk -> (X tile, xoff)
            tts = {}   # blk -> Tt tile
            zb_t = None

            def emit_load(blk):
                nonlocal X
                gs, gsz = g_of[blk]
                if blk != gs:
                    return
                X = xpool.tile([BLK, gsz * NF * D], fp16, tag="X")
                dsrc = dview[gs * BLK : (gs + gsz) * BLK].rearrange(
                    "(t b) d -> b t d", t=gsz
                )
                xd = X.rearrange("b (t c) -> b t c", t=gsz)
                nc.gpsimd.dma_start(out=xd[:, :, 0:D], in_=dsrc)
                esrc = eview[gs * BLK : (gs + gsz) * BLK].rearrange(
                    "(t b) c -> b t c", t=gsz
                )
                nc.gpsimd.dma_start(out=xd[:, :, D:], in_=esrc)

            def emit_transposes(blk):
                """All 27 b->d feature-slab transposes for block blk
                (transpose-mode, fp16 PSUM, cheap DVE/ACT evacuation)."""
                Xb, xoff = xmap[blk]
                Tt = ttpool.tile([128, NF * D], fp16, tag="Tt")
                for ci in range(4):
                    c0 = ci * 8
                    cf = min(8, NF - c0)
                    tp = tppool.tile([128, 8 * BLK], fp16, tag="tp")
                    for j in range(cf):
                        f = c0 + j
                        nc.tensor.transpose(
                            tp[:, j * BLK : (j + 1) * BLK],
                            Xb[:, xoff + f * D : xoff + (f + 1) * D],
                            ident,
                        )
                    dst = Tt[:, c0 * BLK : (c0 + cf) * BLK]
                    src = tp[:, : cf * BLK]
                    if ci % 2 == 0:
                        nc.vector.tensor_copy(out=dst, in_=src)
                    else:
                        nc.scalar.copy(dst, src)
                tts[blk] = Tt

            def emit_grams(blk):
                """128 per-sample gram matmuls for block blk, then the
                DVE StreamTranspose that evacuates Z to SBUF."""
                Tt = tts.pop(blk)
                Ttr = Tt.rearrange("d (f b) -> d b f", b=BLK)
                zp = zppool.tile([128, FP * FP], fp32, tag="zp")
                for q in range(32):
                    for g in range(4):
                        s = q * 4 + g
                        nc.tensor.matmul(
                            zp[32 * g : 32 * g + NF, q * FP : q * FP + NF],
                            Ttr[:, s, :],       # [128 d, 27 f] weights
                            Ttr[:, s, :],       # [128 d, 27 f] moving
                            start=True,
                            stop=True,
                            tile_position=(0, 32 * g),
                        )
                # PSUM [(g,m), (q,n)] -> Zb[(g,q), t, m*32+n]
                t = blk % PACK
                inv = zp.rearrange("p (q n) -> p n q", n=FP)[:, 0:NF, :]
                outv = zb_t.rearrange("p (t m n) -> p t n m", t=PACK, n=FP)[
                    :, t, 0:NF, :
                ]
                nc.vector.transpose(out=outv, in_=inv)

            def emit_pack_out(qtr):
                zbp = zb_t.rearrange("p (t m n) -> p t m n", t=PACK, n=FP)
                Pk = pkpool.tile([128, PACK * NPAIR], fp32, tag="Pk")
                pkp = Pk.rearrange("p (t c) -> p t c", t=PACK)
                off = 0
                for m in range(NF - 1):
                    ln = NF - 1 - m
                    src = zbp[:, :, m, m + 1 : NF]  # [128, t, ln]
                    dst = pkp[:, :, off : off + ln]
                    if m % 2 == 0:
                        nc.vector.tensor_copy(out=dst, in_=src)
                    else:
                        nc.scalar.copy(dst, src)
                    off += ln
                b0 = qtr * PACK * BLK
                ovq = oview[b0 : b0 + PACK * BLK].rearrange(
                    "(t q g) c -> g q t c", t=PACK, g=4
                )  # [4, 32, t, 351]
                pk4 = pkp.rearrange("(g q) t c -> g q t c", g=4)
                for g in range(4):
                    eng = nc.sync if g % 2 == 0 else nc.scalar
                    eng.dma_start(out=ovq[g], in_=pk4[g])

            # Pre-resolve X tiles/offsets (loads happen at group starts).
            def resolve_x(blk):
                gs, gsz = g_of[blk]
                xmap[blk] = (X, (blk - gs) * NF * D)

            # Phase structure: [transpose 8 blocks] [gram 8 blocks] ...
            for ph in range(nblk // PHASE):
                p0 = ph * PHASE
                for blk in range(p0, p0 + PHASE):
                    emit_load(blk)
                    resolve_x(blk)
                    emit_transposes(blk)
                for blk in range(p0, p0 + PHASE):
                    if blk % PACK == 0:
                        zb_t = zbpool.tile(
                            [128, PACK * FP * FP], fp32, tag="Zb"
                        )
                    emit_grams(blk)
                    if blk % PACK == PACK - 1:
                        emit_pack_out(blk // PACK)

    nc.compile()
    return nc


def _get(bc=BC):
    if bc not in _CACHE:
        _CACHE[bc] = build(bc)
    return _CACHE[bc]


def kernel(dense: np.ndarray, embs: np.ndarray) -> np.ndarray:
    from concourse import bass_utils

    dense = np.ascontiguousarray(np.asarray(dense, dtype=np.float32))
    embs = np.ascontiguousarray(np.asarray(embs, dtype=np.float32))
    assert dense.shape == (B, D) and embs.shape == (B, NUM_EMBS, D)

    nc = _get()
    dsh = dense.reshape(N_CORES, BC, D)
    esh = embs.reshape(N_CORES, BC, NUM_EMBS, D)
    in_maps = [{"dense": dsh[i], "embs": esh[i]} for i in range(N_CORES)]
    res = bass_utils.run_bass_kernel_spmd(nc, in_maps, core_ids=list(range(N_CORES)))
    return np.concatenate([r["out"] for r in res.results], axis=0)


# revision 12
# speedup vs baseline: 1.2915x; 1.0321x over previous
"""DLRM dot-interaction kernel for Trainium2 (8 NeuronCores, batch-sharded).

Per sample b: T = concat(dense[b], embs[b]) -> [27, 128]; Z = T @ T^T;
output = strict upper triangle of Z -> [351] fp32.

Per-core plan (2048 samples, 16 blocks of 128):
  - SWDGE cast-DMA loads 4 blocks at a time as [128 b, (f,d)] fp16.
  - Big-phase structure in groups of 8 blocks: first a transpose phase
    (216 PE transpose-mode matmuls [128 b, 128 d] -> PSUM fp16, DVE/ACT
    copy into f-major Tt [128 d, f*128+b] fp16), then one dense gram
    mega-burst (1024 back-to-back per-sample matmuls, ~26us of
    uninterrupted PE work) so the PE HAM activity monitor reaches and
    holds the full 2.4 GHz clock; short interleaved bursts never warm up.
  - Per-sample gram: lhsT = rhs = [128 d, 27 f] strided slice of Tt;
    out -> PSUM zp [32*g + m, q*32 + n] fp32 (col-group tiling,
    sample s = q*4 + g; 4 col groups run concurrently).
  - One DVE StreamTranspose per block swaps m<->q inside each 32-partition
    quadrant (g stays put): PSUM [(g,m), (n,q)-view] -> SBUF Zb
    [(g,q), m*32+n] fp32.  This replaces a DRAM scratch bounce + gather
    (55k small DMA descriptors) with 16 DVE instructions.
  - Triu pack: 26 contiguous-run DVE/ACT copies per 4-block group into
    Pk [(g,q), t*351], then HWDGE DMAs with 1404B runs write out[b, :]
    (partition (g,q) -> row q*4+g).
"""

import numpy as np

B, NUM_EMBS, D = 16384, 26, 128
N_CORES = 8
BC = B // N_CORES  # 2048 samples per core
BLK = 128          # samples per block
NF = NUM_EMBS + 1  # 27 features
FP = 32            # feature pitch in the Z PSUM tile
NPAIR = NF * (NF - 1) // 2  # 351
PACK = 4           # blocks per pack/output group
PHASE = 8          # blocks per transpose/gram phase pair

_CACHE = {}


def build(bc=BC):
    import concourse.bacc as bacc
    import concourse.mybir as mybir
    from concourse.tile import TileContext
    from concourse.masks import make_identity

    fp16 = mybir.dt.float16
    fp32 = mybir.dt.float32

    nc = bacc.Bacc("TRN2", target_bir_lowering=False, debug=False)
    dense_t = nc.dram_tensor("dense", (bc, D), fp32, kind="ExternalInput")
    embs_t = nc.dram_tensor("embs", (bc, NUM_EMBS, D), fp32, kind="ExternalInput")
    out_t = nc.dram_tensor("out", (bc, NPAIR), fp32, kind="ExternalOutput")

    nblk = bc // BLK
    assert nblk % PHASE == 0 and PHASE % PACK == 0

    # Input load groups of 4 blocks (one small head group for fast start).
    groups = []
    b = 0
    head = [1, 3]
    while b < nblk:
        sz = min(head.pop(0) if head else 4, nblk - b)
        groups.append((b, sz))
        b += sz
    g_of = {}
    for gs, sz in groups:
        for i in range(sz):
            g_of[gs + i] = (gs, sz)

    with TileContext(nc) as tc:
        with (
            tc.tile_pool(name="consts", bufs=1) as consts,
            tc.tile_pool(name="xin", bufs=3) as xpool,
            tc.tile_pool(name="tt", bufs=4) as ttpool,
            tc.tile_pool(name="zb", bufs=2) as zbpool,
            tc.tile_pool(name="pk", bufs=2) as pkpool,
            tc.tile_pool(name="tp", bufs=4, space="PSUM") as tppool,
            tc.tile_pool(name="zp", bufs=2, space="PSUM") as zppool,
        ):
            ident = consts.tile([128, 128], fp16)
            make_identity(nc, ident)

            dview = dense_t.ap()  # [bc, 128]
            eview = embs_t.ap().rearrange("b f d -> b (f d)")  # [bc, 3328]
            oview = out_t.ap()  # [bc, 351]

            X = None
            xmap = {}  # blk -> (X tile, xoff)
            tts = {}   # blk -> Tt tile
            zb_t = None

            def emit_load(blk):
                nonlocal X
                gs, gsz = g_of[blk]
                if blk != gs:
                    return
                X = xpool.tile([BLK, gsz * NF * D], fp16, tag="X")
                dsrc = dview[gs * BLK : (gs + gsz) * BLK].rearrange(
                    "(t b) d -> b t d", t=gsz
                )
                xd = X.rearrange("b (t c) -> b t c", t=gsz)
                nc.gpsimd.dma_start(out=xd[:, :, 0:D], in_=dsrc)
                esrc = eview[gs * BLK : (gs + gsz) * BLK].rearrange(
                    "(t b) c -> b t c", t=gsz
                )
                nc.gpsimd.dma_start(out=xd[:, :, D:], in_=esrc)

            def emit_transposes(blk):
                """All 27 b->d feature-slab transposes for block blk
                (transpose-mode, fp16 PSUM, cheap DVE/ACT evacuation)."""
                Xb, xoff = xmap[blk]
                Tt = ttpool.tile([128, NF * D], fp16, tag="Tt")
                for ci in range(4):
                    c0 = ci * 8
                    cf = min(8, NF - c0)
                    tp = tppool.tile([128, 8 * BLK], fp16, tag="tp")
                    for j in range(cf):
                        f = c0 + j
                        nc.tensor.transpose(
                            tp[:, j * BLK : (j + 1) * BLK],
                            Xb[:, xoff + f * D : xoff + (f + 1) * D],
                            ident,
                        )
                    dst = Tt[:, c0 * BLK : (c0 + cf) * BLK]
                    src = tp[:, : cf * BLK]
                    if ci % 2 == 0:
                        nc.vector.tensor_copy(out=dst, in_=src)
                    else:
                        nc.scalar.copy(dst, src)
                tts[blk] = Tt

            def emit_grams(blk):
                """128 per-sample gram matmuls for block blk, then the
                DVE StreamTranspose that evacuates Z to SBUF."""
                Tt = tts.pop(blk)
                Ttr = Tt.rearrange("d (f b) -> d b f", b=BLK)
                zp = zppool.tile([128, FP * FP], fp32, tag="zp")
                for q in range(32):
                    for g in range(4):
                        s = q * 4 + g
                        nc.tensor.matmul(
                            zp[32 * g : 32 * g + NF, q * FP : q * FP + NF],
                            Ttr[:, s, :],       # [128 d, 27 f] weights
                            Ttr[:, s, :],       # [128 d, 27 f] moving
                            start=True,
                            stop=True,
                            tile_position=(0, 32 * g),
                        )
                # PSUM [(g,m), (q,n)] -> Zb[(g,q), t, m*32+n]
                t = blk % PACK
                inv = zp.rearrange("p (q n) -> p n q", n=FP)[:, 0:NF, :]
                outv = zb_t.rearrange("p (t m n) -> p t n m", t=PACK, n=FP)[
                    :, t, 0:NF, :
                ]
                nc.vector.transpose(out=outv, in_=inv)

            def emit_pack_out(qtr):
                zbp = zb_t.rearrange("p (t m n) -> p t m n", t=PACK, n=FP)
                Pk = pkpool.tile([128, PACK * NPAIR], fp32, tag="Pk")
                pkp = Pk.rearrange("p (t c) -> p t c", t=PACK)
                off = 0
                for m in range(NF - 1):
                    ln = NF - 1 - m
                    src = zbp[:, :, m, m + 1 : NF]  # [128, t, ln]
                    dst = pkp[:, :, off : off + ln]
                    # all on ACT: keep DVE free for the StreamTranspose
                    # drain so the gram bursts never stall mid-phase
                    nc.scalar.copy(dst, src)
                    off += ln
                b0 = qtr * PACK * BLK
                ovq = oview[b0 : b0 + PACK * BLK].rearrange(
                    "(t q g) c -> g q t c", t=PACK, g=4
                )  # [4, 32, t, 351]
                pk4 = pkp.rearrange("(g q) t c -> g q t c", g=4)
                for g in range(4):
                    nc.sync.dma_start(out=ovq[g], in_=pk4[g])

            # Pre-resolve X tiles/offsets (loads happen at group starts).
            def resolve_x(blk):
                gs, gsz = g_of[blk]
                xmap[blk] = (X, (blk - gs) * NF * D)

            def emit_gram_block(blk):
                nonlocal zb_t
                if blk % PACK == 0:
                    zb_t = zbpool.tile([128, PACK * FP * FP], fp32, tag="Zb")
                emit_grams(blk)
                if blk % PACK == PACK - 1:
                    emit_pack_out(blk // PACK)

            # Schedule: ignite the PE HAM clock with a 2-block contiguous
            # gram burst (~9us dense regular matmuls -> K flips to 8/8),
            # then sustain with a block-level interleave: each ~1.6us
            # transpose burst is too short for the HAM MID window to
            # re-throttle, and each gram burst renews the busy credit.
            for blk in range(3):
                emit_load(blk)
                resolve_x(blk)
                emit_transposes(blk)
            emit_gram_block(0)
            emit_gram_block(1)
            for blk in range(3, nblk):
                emit_load(blk)
                resolve_x(blk)
                emit_transposes(blk)
                emit_gram_block(blk - 1)
            emit_gram_block(nblk - 1)

    nc.compile()
    return nc


def _get(bc=BC):
    if bc not in _CACHE:
        _CACHE[bc] = build(bc)
    return _CACHE[bc]


def kernel(dense: np.ndarray, embs: np.ndarray) -> np.ndarray:
    from concourse import bass_utils

    dense = np.ascontiguousarray(np.asarray(dense, dtype=np.float32))
    embs = np.ascontiguousarray(np.asarray(embs, dtype=np.float32))
    assert dense.shape == (B, D) and embs.shape == (B, NUM_EMBS, D)

    nc = _get()
    dsh = dense.reshape(N_CORES, BC, D)
    esh = embs.reshape(N_CORES, BC, NUM_EMBS, D)
    in_maps = [{"dense": dsh[i], "embs": esh[i]} for i in range(N_CORES)]
    res = bass_utils.run_bass_kernel_spmd(nc, in_maps, core_ids=list(range(N_CORES)))
    return np.concatenate([r["out"] for r in res.results], axis=0)
